# revision 27
# baseline (speedup 1.0000x reference)
"""ConvCNP encoder kernel for 8 Trainium2 NeuronCores.

Computes, for full inputs X(4,1024,2), Y(4,1024,2), grid(16384,2):
    Gram = exp(-0.5*||grid-X||^2)          (B, G, n)
    FM   = Gram @ [1, Y]                   (B, G, 3)
    out  = [FM0, FM1/FM0, FM2/FM0] -> (B, 3, 128, 128)  (y, x image axes)

The reference grid is a meshgrid, so the 2-D RBF factors into 1-D
Gaussians: Gram[(x,y), n] = A1[x, n] * A2[y, n].

Sharding: core = (batch b, y-half h).  Each core computes its batch
over y in [64h, 64h+64) and ALL 128 x columns; no cross-device
communication.

v3: raw bass (no TileContext).  The NEFF epilogue (walrus-injected
per-semaphore zeroing, ~6.9us paced by the PE sequencer) runs after an
all-engine barrier; measured exec time = last engine's arrival at that
barrier + the fixed epilogue.  So the kernel minimizes barrier arrival:
  - manual semaphores, no TileContext entry/exit overhead (~1.6us less)
  - NOBODY waits on the output DMA completion semaphore: the epilogue
    (~6.9us) far exceeds the DMA latency (~1.6us), so the store always
    lands before NEFF completion.  Drops ~2us of sem-propagation wait.
  - chunks (6,2): after the last exp only 4 mm2 matmuls + norm trail.
  - single input DMA [7, 2240] for all PE factors; E via tiny [128,16]
    DMA with on-device broadcast in the V multiply.

Per-core pipeline:
  mm1 (PE):  A2pre[n,y] = s2[7,n].T @ g2[7,64], A1pre[n,x] likewise,
             chunked (6,2) n-tiles
  exp (ACT): a2c0, a1c0, a2c1, a1c1 (PSUM f32 -> SBUF f16)
  V   (DVE): V[n,t,c,y] = A2[n,t,y] * E[n,t,c]  (E broadcast along y)
  mm2 (PE):  fm0[x,y] += A1.T @ A2 (closes early -> recip overlaps),
             fm12[x,(c,y)] += A1.T @ V
  norm:      reciprocal_approx_fast + fused mul (DVE), density copy (ACT)
  DMA out:   one [128, 384B] f16 store; host reassembles.
"""

import numpy as np

B = 4
N = 1024
G = 16384
NCORES = 8
NT = N // 128             # 8 context tiles
KS = 7                    # 1-D factorization rows
GS = G // NCORES
XCOLS = GS // 128
C0, C1 = 4, 4             # n-tile chunk split

_CACHE = {}


# ---------------------------------------------------------------------------
# v3 separable kernel: raw bass, core = (batch, y-half)
# ---------------------------------------------------------------------------

def _build_nc_v3():
    import concourse.bacc as bacc
    import concourse.mybir as mybir

    f32 = mybir.dt.float32
    f16 = mybir.dt.float16
    EXP = mybir.ActivationFunctionType.Exp

    nc = bacc.Bacc("TRN2", target_bir_lowering=False, debug=False,
                   num_devices=NCORES)
    # in1a: [G2 64 | G1 128 | s2 c0 | s1 c0]; in1b: [s2 c1 | s1 c1]
    # Split across two DMA queues so both halves complete earlier than
    # one serial transfer.
    W1A = 192 + 256 * C0
    W1B = 256 * C1
    in1a_d = nc.dram_tensor("IN1A", [KS, W1A], f16, kind="ExternalInput")
    in1b_d = nc.dram_tensor("IN1B", [KS, W1B], f16, kind="ExternalInput")
    ine_d = nc.dram_tensor("INE", [128, NT * 2], f16, kind="ExternalInput")
    out_d = nc.dram_tensor("OUT", [128, 3, 64], f16, kind="ExternalOutput")

    in1a = nc.alloc_sbuf_tensor("in1a", [KS, W1A], f16)
    in1b = nc.alloc_sbuf_tensor("in1b", [KS, W1B], f16)
    ine = nc.alloc_sbuf_tensor("ine", [128, NT, 2], f16)
    a2sb = nc.alloc_sbuf_tensor("a2sb", [128, NT, 64], f16)
    a1sb = nc.alloc_sbuf_tensor("a1sb", [128, NT, 128], f16)
    vsb = nc.alloc_sbuf_tensor("vsb", [128, NT, 2, 64], f16)
    recip = nc.alloc_sbuf_tensor("recip", [128, 64], f32)
    out_sb = nc.alloc_sbuf_tensor("out_sb", [128, 3, 64], f16)

    # all 8 a2 pre-exponents in ONE psum bank -> a single EXP ACTIVATE
    # covers them (saves one ~270ns ACTIVATE overhead) and both V
    # multiplies unblock right after it.
    ps2 = nc.alloc_psum_tensor("ps2", [128, NT, 64], f32)
    ps1a = nc.alloc_psum_tensor("ps1a", [128, C0, 128], f32)
    ps1b = nc.alloc_psum_tensor("ps1b", [128, C1, 128], f32)
    fm0 = nc.alloc_psum_tensor("fm0", [128, 64], f32)
    # fm12 split into y-halves: the left half's normalization multiply
    # overlaps the right half's accumulation matmuls.
    fm12l = nc.alloc_psum_tensor("fm12l", [128, 2, 32], f32)
    fm12r = nc.alloc_psum_tensor("fm12r", [128, 2, 32], f32)

    s_in1a = nc.alloc_semaphore("s_in1a", num=208)
    s_in1b = nc.alloc_semaphore("s_in1b", num=209)
    s_pe = nc.alloc_semaphore("s_pe", num=210)
    s_act = nc.alloc_semaphore("s_act", num=211)
    s_dve = nc.alloc_semaphore("s_dve", num=212)
    s_out = nc.alloc_semaphore("s_out", num=213)
    s_ine = nc.alloc_semaphore("s_ine", num=214)

    g2 = in1a[0:KS, 0:64]
    g1 = in1a[0:KS, 64:192]

    def s2_sl(t):
        if t < C0:
            return in1a[0:KS, 192 + 128 * t: 192 + 128 * (t + 1)]
        return in1b[0:KS, 128 * (t - C0): 128 * (t - C0 + 1)]

    def s1_sl(t):
        if t < C0:
            off = 192 + 128 * C0
            return in1a[0:KS, off + 128 * t: off + 128 * (t + 1)]
        off = 128 * C1
        return in1b[0:KS, off + 128 * (t - C0): off + 128 * (t - C0 + 1)]

    # --- input DMAs.  GpSimd's FIRST dma_start has no pre-DRAIN and a
    # ~650ns issue (vs Sync's ~950), so the mm1-gating in1a goes there.
    # Sync takes in1b then ine (needed later); Scalar stays free so its
    # ACT_TABLE_LOAD starts immediately.
    nc.gpsimd.dma_start(in1a.ap(), in1a_d[:, :]).then_inc(s_in1a, 16)
    nc.sync.dma_start(in1b.ap(), in1b_d[:, :]).then_inc(s_in1b, 16)
    nc.sync.dma_start(ine.ap(), ine_d[:, :]).then_inc(s_ine, 16)

    # --- mm1: all 8 a2 tiles first (fills the single ps2 bank), then a1
    # chunk 0, then a1 chunk 1 ---
    nc.tensor.wait_ge(s_in1a, 16)
    for i in range(C0):
        nc.tensor.matmul(ps2[:, i, :], s2_sl(i), g2,
                         start=True, stop=True).then_inc(s_pe)
    nc.tensor.wait_ge(s_in1b, 16)
    for i in range(C1):
        nc.tensor.matmul(ps2[:, C0 + i, :], s2_sl(C0 + i), g2,
                         start=True, stop=True).then_inc(s_pe)
    for i in range(C0):
        nc.tensor.matmul(ps1a[:, i, :], s1_sl(i), g1,
                         start=True, stop=True).then_inc(s_pe)
    for i in range(C1):
        nc.tensor.matmul(ps1b[:, i, :], s1_sl(C0 + i), g1,
                         start=True, stop=True).then_inc(s_pe)

    # --- exp chain on ACT: 3 ACTIVATEs (a2-all, a1c0, a1c1) ---
    nc.scalar.wait_ge(s_pe, NT)
    nc.scalar.activation(out=a2sb.ap(), in_=ps2.ap(),
                         func=EXP).then_inc(s_act)           # s_act=1
    nc.scalar.wait_ge(s_pe, NT + C0)
    nc.scalar.activation(out=a1sb[:, 0:C0, :], in_=ps1a.ap(),
                         func=EXP).then_inc(s_act)           # s_act=2
    nc.scalar.wait_ge(s_pe, 2 * NT)
    nc.scalar.activation(out=a1sb[:, C0:NT, :], in_=ps1b.ap(),
                         func=EXP).then_inc(s_act)           # s_act=3

    # --- V on DVE: V[n,t,c,y] = A2[n,t,y] * E[n,t,c]; both halves
    # unblock after the single a2 exp ---
    nc.vector.wait_ge(s_act, 1)
    nc.vector.wait_ge(s_ine, 16)
    a2b0 = a2sb[:, 0:C0, None, :].to_broadcast([128, C0, 2, 64])
    e0 = ine[:, 0:C0, :, None].to_broadcast([128, C0, 2, 64])
    nc.vector.tensor_mul(vsb[:, 0:C0, :, :], a2b0, e0).then_inc(s_dve)  # 1
    a2b1 = a2sb[:, C0:NT, None, :].to_broadcast([128, C1, 2, 64])
    e1 = ine[:, C0:NT, :, None].to_broadcast([128, C1, 2, 64])
    nc.vector.tensor_mul(vsb[:, C0:NT, :, :], a2b1, e1).then_inc(s_dve)  # 2

    # --- mm2 ---
    # s_pe counts: mm1 = 16; per chunk fm0 then fm12l then fm12r.
    # fm0 closes at 16+C0+2*C0+C1 = 28 (with (4,4): 28); fm12l at 36;
    # fm12r at 40.
    nc.tensor.wait_ge(s_act, 2)
    for i in range(C0):
        nc.tensor.matmul(fm0.ap(), a1sb[:, i, :], a2sb[:, i, :],
                         start=(i == 0), stop=False).then_inc(s_pe)
    nc.tensor.wait_ge(s_dve, 1)
    for i in range(C0):
        # l/r share lhsT per tile -> one LDWEIGHTS for the pair
        nc.tensor.matmul(fm12l.ap(), a1sb[:, i, :], vsb[:, i, :, 0:32],
                         start=(i == 0), stop=False).then_inc(s_pe)
        nc.tensor.matmul(fm12r.ap(), a1sb[:, i, :], vsb[:, i, :, 32:64],
                         start=(i == 0), stop=False).then_inc(s_pe)
    nc.tensor.wait_ge(s_act, 3)
    for i in range(C1):
        nc.tensor.matmul(fm0.ap(), a1sb[:, C0 + i, :], a2sb[:, C0 + i, :],
                         start=False, stop=(i == C1 - 1)).then_inc(s_pe)
    nc.tensor.wait_ge(s_dve, 2)
    # c1 keeps l-block then r-block so fm12l closes 4 matmuls early and
    # its normalization mul overlaps the r-block.
    for i in range(C1):
        nc.tensor.matmul(fm12l.ap(), a1sb[:, C0 + i, :],
                         vsb[:, C0 + i, :, 0:32],
                         start=False, stop=(i == C1 - 1)).then_inc(s_pe)
    for i in range(C1):
        nc.tensor.matmul(fm12r.ap(), a1sb[:, C0 + i, :],
                         vsb[:, C0 + i, :, 32:64],
                         start=False, stop=(i == C1 - 1)).then_inc(s_pe)

    n_fm0_close = 2 * NT + 3 * C0 + C1                 # 32 (after fm0-c1)
    n_fm12l_close = n_fm0_close + C1                   # 36 (after l-c1)
    n_all = 2 * NT + 3 * NT                            # 40

    # --- normalization ---
    nc.vector.wait_ge(s_pe, n_fm0_close)
    nc.vector.reciprocal_approx_fast(out=recip.ap(), in_=fm0.ap()).then_inc(s_dve)  # 3
    nc.scalar.wait_ge(s_pe, n_fm0_close)
    nc.scalar.copy(out_sb[:, 0, :], fm0.ap()).then_inc(s_act)            # 4
    nc.vector.wait_ge(s_pe, n_fm12l_close)
    rec_l = recip[:, None, 0:32].to_broadcast([128, 2, 32])
    nc.vector.tensor_mul(out_sb[:, 1:3, 0:32], fm12l.ap(), rec_l).then_inc(s_dve)  # 4
    nc.vector.wait_ge(s_pe, n_all)
    rec_r = recip[:, None, 32:64].to_broadcast([128, 2, 32])
    nc.vector.tensor_mul(out_sb[:, 1:3, 32:64], fm12r.ap(), rec_r).then_inc(s_dve)  # 5

    # --- output DMA: issued once norm lands; completion is NOT waited on
    # (the fixed NEFF epilogue outlasts the DMA latency).  Only
    # SP/Activation/GpSimd can issue DMAs; Sync's HWDGE has no pre-DRAIN.
    nc.sync.wait_ge(s_act, 4)
    nc.sync.wait_ge(s_dve, 5)
    nc.sync.dma_start(out_d[:, :, :], out_sb.ap()).then_inc(s_out, 16)

    nc.compile()
    return nc


def _sep_factors(gv, xc):
    """K=7 fp16 factorization of -0.5 (g - x)^2 along one dimension.
    xc: (N,) context coords, gv: (M,) grid coords.
    Returns A [7, N] stationary rows and Bm [7, M] moving rows."""
    f16 = np.float16

    def split(a):
        hi = a.astype(f16).astype(np.float32)
        lo = (a - hi).astype(f16).astype(np.float32)
        return hi, lo

    sx = -0.5 * xc * xc
    sg = -0.5 * gv * gv
    xh, xl = split(xc)
    gh, gl = split(gv)
    sxh, sxl = split(sx)
    sgh, sgl = split(sg)
    on = np.ones_like(xc)
    og = np.ones_like(gv)
    A = np.stack([xh, xl, xh, sxh, sxl, on, on], axis=0)
    Bm = np.stack([gh, gh, gl, og, og, sgh, sgl], axis=0)
    return A.astype(f16), Bm.astype(f16)


def _prepare_inputs_v3(X, Y, grid):
    f16 = np.float16
    X = np.asarray(X, np.float32)
    Y = np.asarray(Y, np.float32)
    grid = np.asarray(grid, np.float32)
    gxv = grid[::128, 0]
    gyv = grid[:128, 1]

    in_maps = []
    for c in range(NCORES):
        b, h = divmod(c, 2)
        A1s, G1 = _sep_factors(gxv, X[b, :, 0])                 # [7,1024],[7,128]
        A2s, G2 = _sep_factors(gyv[64 * h:64 * h + 64], X[b, :, 1])  # [7,1024],[7,64]
        cut = 128 * C0
        in1a = np.concatenate([G2, G1, A2s[:, :cut], A1s[:, :cut]], axis=1)
        in1b = np.concatenate([A2s[:, cut:], A1s[:, cut:]], axis=1)
        ine = Y[b].reshape(NT, 128, 2).transpose(1, 0, 2).reshape(128, NT * 2)
        in_maps.append({"IN1A": np.ascontiguousarray(in1a).astype(f16),
                        "IN1B": np.ascontiguousarray(in1b).astype(f16),
                        "INE": np.ascontiguousarray(ine).astype(f16)})
    return in_maps


def _grid_separable(grid):
    grid = np.asarray(grid)
    if grid.shape != (G, 2):
        return False
    gxv = grid[::128, 0]
    gyv = grid[:128, 1]
    return (np.array_equal(grid[:, 0], np.repeat(gxv, 128))
            and np.array_equal(grid[:, 1], np.tile(gyv, 128)))


# ---------------------------------------------------------------------------
# General fallback (grid not a meshgrid): grid-axis shard, K=10 bf16
# hi/lo-split factorization of -0.5*d^2.  Unused for the reference grid.
# ---------------------------------------------------------------------------

NTG = N // 128
JS = GS // 512
K = 10
A_W = B * 2 * 128
B_W = GS
E_W = B * NTG * 5
IN_W = A_W + B_W + E_W


def _build_nc_general():
    import concourse.bacc as bacc
    import concourse.mybir as mybir
    import concourse.tile as tile
    from contextlib import ExitStack

    f32 = mybir.dt.float32
    bf16 = mybir.dt.bfloat16

    nc = bacc.Bacc("TRN2", target_bir_lowering=False, debug=False,
                   num_devices=NCORES)
    in_d = nc.dram_tensor("IN", [128, IN_W], bf16, kind="ExternalInput")
    out_d = nc.dram_tensor("OUT", [B, 3, 128, XCOLS], f32, kind="ExternalOutput")

    EXP = mybir.ActivationFunctionType.Exp

    with tile.TileContext(nc) as tc, ExitStack() as ctx:
        consts = ctx.enter_context(tc.tile_pool(name="consts", bufs=1))
        gram_pool = ctx.enter_context(tc.tile_pool(name="gram", bufs=4))
        mm1a_pool = ctx.enter_context(tc.tile_pool(name="mm1a", bufs=1, space="PSUM"))
        mm1b_pool = ctx.enter_context(tc.tile_pool(name="mm1b", bufs=1, space="PSUM"))
        mm2_pool = ctx.enter_context(tc.tile_pool(name="mm2", bufs=1, space="PSUM"))
        small = ctx.enter_context(tc.tile_pool(name="small", bufs=4))
        outp = ctx.enter_context(tc.tile_pool(name="outp", bufs=1))

        a0_sb = consts.tile([128, 2 * 128], bf16)
        a123_sb = consts.tile([128, 3 * 2 * 128], bf16)
        b_t = [consts.tile([128, 512], bf16, name=f"bj{j}", tag=f"bj{j}")
               for j in range(JS)]
        e_sb = consts.tile([128, E_W], bf16)

        def in_col(c0, w):
            return in_d[:, c0:c0 + w]

        nc.sync.dma_start(out=a0_sb, in_=in_col(0, 256))
        nc.sync.dma_start(out=b_t[0], in_=in_col(256, 512))
        nc.gpsimd.dma_start(out=b_t[1], in_=in_col(768, 512))
        nc.sync.dma_start(out=a123_sb, in_=in_col(1280, 768))
        nc.gpsimd.dma_start(out=b_t[2], in_=in_col(2048, 512))
        nc.sync.dma_start(out=b_t[3], in_=in_col(2560, 512))
        nc.gpsimd.dma_start(out=e_sb, in_=in_col(3072, E_W))

        a0_v = a0_sb.rearrange("p (h m) -> p h m", h=2)
        a123_v = a123_sb.rearrange("p (b h m) -> p b h m", b=3, h=2)
        e_v = e_sb.rearrange("p (b t c) -> p b t c", b=B, t=NTG)

        def a_slice(b, row, h4):
            if b == 0:
                return a0_v[32 * row:32 * row + K, h4, :]
            return a123_v[32 * row:32 * row + K, b - 1, h4, :]

        def b_slice(j):
            return b_t[j]

        out_sb = outp.tile([128, B, 3, XCOLS], f32)
        grams = {}

        def emit_mm1_group(b, slots, ps):
            for i, s in enumerate(slots):
                j = s // 8
                nt = s % 8
                row = nt % 4
                lhsT = a_slice(b, row, nt // 4)
                rhs = b_slice(j)[32 * row:32 * row + K, :]
                nc.tensor.matmul(ps[:, i, :], lhsT, rhs,
                                 start=True, stop=True,
                                 tile_position=(32 * row, 0))

        def emit_mm1_exp(b, h, sizes, tags):
            gram = gram_pool.tile([128, 16, 512], bf16, tag="gram",
                                  name=f"gram{b}{h}")
            grams[(b, h)] = gram
            s0 = 0
            for gsz, sel in zip(sizes, tags):
                pool = (mm1a_pool, mm1b_pool)[sel]
                cap = (4, 3)[sel]
                ps = pool.tile([128, cap, 512], f32, tag=f"t{sel}",
                               name=f"ps{sel}")
                emit_mm1_group(b, [16 * h + s0 + i for i in range(gsz)], ps)
                nc.scalar.activation(out=gram[:, s0:s0 + gsz, :],
                                     in_=ps[:, 0:gsz, :], func=EXP)
                s0 += gsz

        def emit_mm2_j(b, j, gram, base):
            fm = grams[("fm", b)]
            for r in range(4):
                gsub = j * 4 + r
                for nt in range(NTG):
                    nc.tensor.matmul(
                        fm[:, gsub, :],
                        gram[:, base + nt, r * 128:(r + 1) * 128],
                        e_v[:, b, nt, :],
                        start=(nt == 0),
                        stop=(nt == NTG - 1),
                    )

        def emit_norm(b, sl, dma_engine):
            fm = grams[("fm", b)]
            w = sl.stop - sl.start
            fmc = small.tile([128, 8, 5], f32, tag="fmc")
            nc.vector.tensor_copy(fmc[:, 0:w, :], fm[:, sl, :])
            recip = small.tile([128, 8], f32, tag="recip")
            nc.vector.reciprocal(recip[:, 0:w], fmc[:, 0:w, 0])
            nc.vector.tensor_copy(out_sb[:, b, 0, sl], fmc[:, 0:w, 0])
            v1 = small.tile([128, 8], f32, tag="v1")
            nc.vector.tensor_add(v1[:, 0:w], fmc[:, 0:w, 1], fmc[:, 0:w, 3])
            nc.vector.tensor_mul(out_sb[:, b, 1, sl], v1[:, 0:w], recip[:, 0:w])
            v2 = small.tile([128, 8], f32, tag="v2")
            nc.vector.tensor_add(v2[:, 0:w], fmc[:, 0:w, 2], fmc[:, 0:w, 4])
            nc.vector.tensor_mul(out_sb[:, b, 2, sl], v2[:, 0:w], recip[:, 0:w])
            dst = out_d[b, :, :, sl].rearrange("c y x -> y c x")
            dma_engine.dma_start(out=dst, in_=out_sb[:, b, :, sl])

        half_patterns = [((2, 4, 3, 4, 3), (1, 0, 1, 0, 1))]
        for k in range(1, 7):
            if k % 2 == 1:
                half_patterns.append(((4, 3, 4, 3, 2), (0, 1, 0, 1, 0)))
            else:
                half_patterns.append(((3, 4, 3, 4, 2), (1, 0, 1, 0, 1)))

        for b in range(B):
            fm_t = mm2_pool.tile([128, XCOLS, 5], f32, tag="fm")
            grams[("fm", b)] = fm_t
            if b < B - 1:
                emit_mm1_exp(b, 0, *half_patterns[2 * b])
                emit_mm1_exp(b, 1, *half_patterns[2 * b + 1])
            else:
                emit_mm1_exp(b, 0, *half_patterns[6])
                emit_mm1_exp(b, 1, ((4, 3, 4, 3, 2)), ((0, 1, 0, 1, 0)))
            if b >= 1:
                p = b - 1
                for h in range(2):
                    g = grams[(p, h)]
                    emit_mm2_j(p, 2 * h, g, 0)
                    emit_mm2_j(p, 2 * h + 1, g, 8)
                    emit_norm(p, slice(8 * h, 8 * h + 8), nc.sync)
        b = B - 1
        for h in range(2):
            g = grams[(b, h)]
            emit_mm2_j(b, 2 * h, g, 0)
            emit_mm2_j(b, 2 * h + 1, g, 8)
            emit_norm(b, slice(8 * h, 8 * h + 8), nc.sync)

    nc.compile()
    return nc


def _split_hi_lo(a):
    import ml_dtypes

    bf = ml_dtypes.bfloat16
    hi = a.astype(bf).astype(np.float32)
    lo = (a - hi).astype(bf).astype(np.float32)
    return hi, lo


def _prepare_inputs(X, Y, grid):
    import ml_dtypes

    bf = ml_dtypes.bfloat16
    X = np.asarray(X, np.float32)
    Y = np.asarray(Y, np.float32)
    grid = np.asarray(grid, np.float32)

    sx = -0.5 * np.sum(X * X, axis=-1)
    sg = -0.5 * np.sum(grid * grid, axis=-1)
    xh, xl = _split_hi_lo(X)
    gh, gl = _split_hi_lo(grid)
    sxh, sxl = _split_hi_lo(sx)
    sgh, sgl = _split_hi_lo(sg)
    ones_n = np.ones((B, N), np.float32)
    ones_g = np.ones((G,), np.float32)

    A = np.stack(
        [xh[..., 0], xh[..., 1], xl[..., 0], xl[..., 1],
         xh[..., 0], xh[..., 1], sxh, sxl, ones_n, ones_n],
        axis=1,
    )
    Bm = np.stack(
        [gh[:, 0], gh[:, 1], gh[:, 0], gh[:, 1],
         gl[:, 0], gl[:, 1], ones_g, ones_g, sgh, sgl],
        axis=0,
    )

    A4 = A.transpose(1, 0, 2).reshape(K, B, 2, 4, 128)
    arep = np.zeros((128, B, 2, 128), np.float32)
    for i in range(4):
        arep[32 * i:32 * i + K] = A4[:, :, :, i, :]

    yh, yl = _split_hi_lo(Y)
    E = np.stack([ones_n, yh[..., 0], yh[..., 1], yl[..., 0], yl[..., 1]],
                 axis=-1)
    ey = E.reshape(B, NTG, 128, 5).transpose(2, 0, 1, 3)

    in_maps = []
    ar = arep.reshape(128, A_W)
    for c in range(NCORES):
        brep = np.zeros((128, GS), np.float32)
        for i in range(4):
            brep[32 * i:32 * i + K] = Bm[:, c * GS:(c + 1) * GS]
        packed = np.concatenate(
            [ar[:, 0:256], brep[:, 0:512], brep[:, 512:1024],
             ar[:, 256:A_W], brep[:, 1024:1536], brep[:, 1536:2048],
             ey.reshape(128, E_W)], axis=1)
        in_maps.append({"IN": np.ascontiguousarray(packed).astype(bf)})
    return in_maps


def _run(in_maps, builder, key, trace=False):
    from concourse.bass_utils import run_bass_kernel_spmd

    if key not in _CACHE:
        _CACHE[key] = builder()
    nc = _CACHE[key]
    return run_bass_kernel_spmd(nc, in_maps, core_ids=list(range(NCORES)),
                                trace=trace)


def kernel(X, Y, grid, _trace=False, _results_out=None):
    out = np.empty((B, 3, 128, 128), np.float32)
    if _grid_separable(grid):
        in_maps = _prepare_inputs_v3(X, Y, grid)
        res = _run(in_maps, _build_nc_v3, "v3", trace=_trace)
        for c in range(NCORES):
            b, h = divmod(c, 2)
            o = res.results[c]["OUT"].astype(np.float32)   # (128x, 3, 64y)
            out[b, :, 64 * h:64 * h + 64, :] = o.transpose(1, 2, 0)
    else:
        in_maps = _prepare_inputs(X, Y, grid)
        res = _run(in_maps, _build_nc_general, "gen", trace=_trace)
        for c in range(NCORES):
            out[:, :, :, c * XCOLS:(c + 1) * XCOLS] = res.results[c]["OUT"]
    if _results_out is not None:
        _results_out.append(res)
    return out
